# revision 1
# baseline (speedup 1.0000x reference)
"""Trainium2 Bass kernel for nn_DefuzzyLayer: out = x @ rules_outcome.

x: [8192, 4096] f32, rules_outcome: [4096, 4096] f32 -> out: [8192, 4096] f32.

Strategy: data-parallel over batch. Each of the 8 NeuronCores computes a
[1024, 4096] output shard: x_shard @ W with the full W replicated.

Per-core kernel (Tile framework):
  - Inputs are cast to fp16 host-side (PSUM accumulation stays fp32, so the
    only loss is input rounding: ~3e-4 relative error on the output).
  - Host pre-packs x^T and W into partition-major SBUF layout so every load
    is a fully-contiguous multi-MiB DMA (one for each x quarter, one per W
    n-block); stores batch one [1024, 512] block per DMA on the scalar
    HWDGE ring, separate from the load ring on the sync engine.
  - x shard stays fully resident in SBUF (64 KiB/partition in fp16);
    W streams through 2 double-buffered block tiles.
  - Loop: n-block (512 cols) outer, k inner, m innermost; each n-block
    accumulates 8 m-tiles into the 8 PSUM banks over 32 k-tiles, then
    evicts PSUM -> SBUF -> DRAM.
"""

import numpy as np

BATCH = 8192
IN_DIM = 4096
OUT_DIM = 4096
N_CORES = 8
M_SHARD = BATCH // N_CORES  # 1024

P = 128
NB = int(__import__("os").environ.get("KNB", "256"))  # moving free-dim per matmul
KT = IN_DIM // P            # 32 k-tiles
MT = M_SHARD // P           # 8 m-tiles
NBLK = OUT_DIM // NB        # 8 n-blocks
XCHUNKS = int(__import__("os").environ.get("KXC", "8"))  # x load split
KPC = KT // XCHUNKS         # k-tiles per x chunk
ORDER = __import__("os").environ.get("KORDER", "mi")  # "mi": k outer/m inner; "ki": m outer/k inner
PS_BUFS = int(__import__("os").environ.get("KPSBUFS", "8"))

IN_DT = __import__("os").environ.get("KDT", "float16")  # float32r | float16 | bfloat16

_cached_nc = None


def _np_dt():
    if IN_DT == "float16":
        return np.float16
    if IN_DT == "bfloat16":
        import ml_dtypes
        return np.dtype(ml_dtypes.bfloat16)
    return np.float32


def _build(loop_n=1, in_dt=None, variant="full"):
    """Build + compile the per-core Bass module.

    loop_n > 1 wraps the whole body in an on-device For_i loop — used only
    for HW timing (amortizes host dispatch overhead out of the measurement).
    variant: "full" | "nodma" (skip x/w loads) | "dmaonly" (skip compute).
    """
    import contextlib
    import concourse.bacc as bacc
    import concourse.tile as tile
    import concourse.mybir as mybir

    do_in_dma = variant not in ("nodma", "mmonly")
    do_compute = variant != "dmaonly"
    do_evict = variant != "mmonly"

    dt_in = getattr(mybir.dt, in_dt or IN_DT)

    nc = bacc.Bacc("TRN2", target_bir_lowering=False, debug=False)
    # partition-major packed inputs (see _pack_x_shard/_pack_w)
    xt = nc.dram_tensor(
        "xt", [P, KT * M_SHARD], dt_in, kind="ExternalInput"
    ).ap()
    w = nc.dram_tensor(
        "w", [P, NBLK * KT * NB], dt_in, kind="ExternalInput"
    ).ap()
    out = nc.dram_tensor(
        "out", [M_SHARD, OUT_DIM], mybir.dt.float32, kind="ExternalOutput"
    ).ap()
    out_r = out.rearrange("(m p) n -> p m n", p=P)  # [128, MT, OUT_DIM]

    with tile.TileContext(nc) as tc:
        loop_ctx = (
            tc.For_i(0, loop_n, 1,
                     hint_engines=(mybir.EngineType.PE, mybir.EngineType.SP,
                                   mybir.EngineType.DVE))
            if loop_n > 1 else contextlib.nullcontext()
        )
        with (
            loop_ctx,
            tc.tile_pool(name="xpool", bufs=XCHUNKS) as xpool,
            tc.tile_pool(name="wpool", bufs=int(__import__("os").environ.get("KWB", "3"))) as wpool,
            tc.tile_pool(name="opool", bufs=2) as opool,
            tc.tile_pool(name="pspool", bufs=PS_BUFS, space="PSUM") as pspool,
        ):
            x_chunks = []
            for c in range(XCHUNKS):
                x_c = xpool.tile([P, KPC * M_SHARD], dt_in,
                                 name=f"x{c}", tag="x")
                if do_in_dma:
                    nc.sync.dma_start(
                        out=x_c[:],
                        in_=xt[:, c * KPC * M_SHARD:(c + 1) * KPC * M_SHARD],
                    )
                else:
                    nc.vector.memset(x_c[:, 0:1], 0.0)
                x_chunks.append(x_c)

            shared_psums = None
            if not do_evict:
                shared_psums = [
                    pspool.tile([P, NB], mybir.dt.float32,
                                name=f"sps{m}", tag="ps")
                    for m in range(MT)
                ]
            for b in range(NBLK):
                w_b = wpool.tile([P, KT * NB], dt_in, name=f"w{b}", tag="w")
                if do_in_dma:
                    nc.sync.dma_start(
                        out=w_b[:],
                        in_=w[:, b * KT * NB:(b + 1) * KT * NB],
                    )
                else:
                    nc.vector.memset(w_b[:, 0:1], 0.0)

                if not do_compute:
                    continue
                o_b = None
                if do_evict:
                    o_b = opool.tile([P, MT, NB], mybir.dt.float32,
                                     name=f"o{b}", tag="o")
                if ORDER == "mi":
                    if shared_psums is not None:
                        psums = shared_psums
                    else:
                        psums = [
                            pspool.tile([P, NB], mybir.dt.float32,
                                        name=f"ps{b}_{m}", tag="ps")
                            for m in range(MT)
                        ]
                    for k in range(KT):
                        xc = x_chunks[k // KPC]
                        koff = (k % KPC) * M_SHARD
                        for m in range(MT):
                            nc.tensor.matmul(
                                psums[m][:],
                                xc[:, koff + m * P:koff + (m + 1) * P],
                                w_b[:, k * NB:(k + 1) * NB],
                                start=(k == 0),
                                stop=(k == KT - 1),
                            )
                    if do_evict:
                        for m in range(MT):
                            nc.vector.tensor_copy(o_b[:, m, :], psums[m][:])
                else:  # "ki": bank-dwell — one psum bank through all k
                    for m in range(MT):
                        ps = pspool.tile([P, NB], mybir.dt.float32,
                                         name=f"ps{b}_{m}", tag="ps")
                        for k in range(KT):
                            xc = x_chunks[k // KPC]
                            koff = (k % KPC) * M_SHARD
                            nc.tensor.matmul(
                                ps[:],
                                xc[:, koff + m * P:koff + (m + 1) * P],
                                w_b[:, k * NB:(k + 1) * NB],
                                start=(k == 0),
                                stop=(k == KT - 1),
                            )
                        nc.vector.tensor_copy(o_b[:, m, :], ps[:])
                if do_evict:
                    nc.scalar.dma_start(
                        out=out_r[:, :, b * NB:(b + 1) * NB],
                        in_=o_b[:],
                    )

    nc.compile()
    return nc


def _get_nc():
    global _cached_nc
    if _cached_nc is None:
        _cached_nc = _build()
    return _cached_nc


def _pack_x_shard(x_shard):
    """[M_SHARD, IN_DIM] -> [128, KT*M_SHARD] partition-major."""
    # dest[p, k*M_SHARD + m] = x_shard[m, k*128 + p]
    return np.ascontiguousarray(
        x_shard.T.reshape(KT, P, M_SHARD).transpose(1, 0, 2).reshape(P, -1)
    )


def _pack_w(w_full):
    """[IN_DIM, OUT_DIM] -> [128, NBLK*KT*NB] partition-major."""
    # dest[p, b*(KT*NB) + k*NB + j] = w_full[k*128 + p, b*NB + j]
    return np.ascontiguousarray(
        w_full.reshape(KT, P, NBLK, NB).transpose(1, 2, 0, 3).reshape(P, -1)
    )


def _make_in_maps(x, rules_outcome):
    np_dt = _np_dt()
    x = np.asarray(x, dtype=np_dt)
    w = np.asarray(rules_outcome, dtype=np_dt)
    assert x.shape == (BATCH, IN_DIM) and w.shape == (IN_DIM, OUT_DIM)
    w_packed = _pack_w(w)
    return [
        {
            "xt": _pack_x_shard(x[i * M_SHARD:(i + 1) * M_SHARD, :]),
            "w": w_packed,
        }
        for i in range(N_CORES)
    ]


def _run(x, rules_outcome, **spmd_kwargs):
    from concourse.bass_utils import run_bass_kernel_spmd

    in_maps = _make_in_maps(x, rules_outcome)
    nc = _get_nc()
    res = run_bass_kernel_spmd(nc, in_maps, core_ids=list(range(N_CORES)),
                               **spmd_kwargs)
    full = np.concatenate([res.results[i]["out"] for i in range(N_CORES)],
                          axis=0)
    return full, res


def kernel(x, rules_outcome):
    out, _ = _run(x, rules_outcome)
    return out



# revision 14
# speedup vs baseline: 1.2860x; 1.2860x over previous
"""Trainium2 Bass kernel for nn_DefuzzyLayer: out = x @ rules_outcome.

x: [8192, 4096] f32, rules_outcome: [4096, 4096] f32 -> out: [8192, 4096] f32.

Strategy: data-parallel over batch. Each of the 8 NeuronCores computes a
[1024, 4096] output shard: x_shard @ W with the full W replicated.

Per-core kernel (Tile framework):
  - Inputs are cast to fp16 host-side (PSUM accumulation stays fp32, so the
    only loss is input rounding: ~3e-4 relative error on the output).
  - Host pre-packs x^T and W into partition-major SBUF layout so every load
    is a fully-contiguous multi-MiB DMA. Ring split: W loads on the sync
    HWDGE ring; x loads + output stores on the scalar/ACT ring (measured
    -20us vs everything on one ring).
  - x shard is fully double-buffered in SBUF (16 chunk bufs) so back-to-back
    invocations overlap x reload with tail compute; W streams through
    triple-buffered block tiles. Outputs stage through SBUF as bf16 (halves
    store traffic; adds ~1e-3 rel err, well under the 2e-2 gate) and are
    upcast to f32 on the host.
  - Orders:
      "mi": n-block outer, k middle, m inner (8 PSUM banks by m-tile).
      "ki": bank-dwell, one psum bank through all k.
      "nd": GW n-blocks processed jointly; inner (m, g) loop issues GW
            consecutive matmuls sharing the same stationary x-tile
            (stationary-dwell), 8 PSUM banks = MCH m-tiles x GW blocks.
"""

import numpy as np

BATCH = 8192
IN_DIM = 4096
OUT_DIM = 4096
N_CORES = 8
M_SHARD = BATCH // N_CORES  # 1024

_env = __import__("os").environ
P = 128
ORDER = _env.get("KORDER", "mi")  # "mi" | "ki" | "nd"
NB = int(_env.get("KNB", "256"))  # moving free-dim per matmul
KT = IN_DIM // P            # 32 k-tiles
MT = M_SHARD // P           # 8 m-tiles
NBLK = OUT_DIM // NB        # n-blocks
XCHUNKS = int(_env.get("KXC", "8"))  # x load split (by k)
KPC = KT // XCHUNKS         # k-tiles per x chunk
XB = int(_env.get("KXB", "16"))  # x pool bufs (2x XCHUNKS = full double buffer)
PS_BUFS = int(_env.get("KPSBUFS", "8"))
GW = int(_env.get("KGW", "2"))   # "nd": n-blocks grouped (stationary dwell)
MCH = int(_env.get("KMCH", str(max(1, 8 // max(GW, 1)))))  # "nd": m-tiles in flight
WB = int(_env.get("KWB", "3"))
OB = int(_env.get("KOB", "4"))
EV = _env.get("KEV", "v")  # "v": vector evictions; "vs": alternate vector/scalar
XLAYOUT = _env.get("KXL", "k")  # "k": x chunked by k (resident); "m": by m-tile (streamed)
LDW = _env.get("KLDW", "0") == "1"  # "nd": explicit ldweights per dwell group
XRING = _env.get("KXRING", "act")  # engine ring for x loads: sync | act
WRING = _env.get("KWRING", "sync")  # engine ring for w loads: sync | split
ODT = _env.get("KODT", "bfloat16")  # output staging/DMA dtype: float32 | bfloat16

IN_DT = _env.get("KDT", "float16")  # float32r | float16 | bfloat16

_cached_nc = None


def _np_dt():
    if IN_DT == "float16":
        return np.float16
    if IN_DT == "bfloat16":
        import ml_dtypes
        return np.dtype(ml_dtypes.bfloat16)
    return np.float32


def _build(loop_n=1, in_dt=None, variant="full"):
    """Build + compile the per-core Bass module.

    loop_n > 1 wraps the whole body in an on-device For_i loop — used only
    for HW timing (amortizes host dispatch overhead out of the measurement).
    variant: "full" | "nodma" (skip x/w loads) | "dmaonly" (skip compute)
             | "mmonly" (skip loads + evictions).
    """
    import contextlib
    import concourse.bacc as bacc
    import concourse.tile as tile
    import concourse.mybir as mybir

    do_in_dma = variant not in ("nodma", "mmonly")
    do_compute = variant != "dmaonly"
    do_evict = variant != "mmonly"

    dt_in = getattr(mybir.dt, in_dt or IN_DT)

    nc = bacc.Bacc("TRN2", target_bir_lowering=False, debug=False)
    # partition-major packed inputs (see _pack_x_shard/_pack_w)
    xt = nc.dram_tensor(
        "xt", [P, KT * M_SHARD], dt_in, kind="ExternalInput"
    ).ap()
    w = nc.dram_tensor(
        "w", [P, NBLK * KT * NB], dt_in, kind="ExternalInput"
    ).ap()
    dt_out = getattr(mybir.dt, ODT)
    out = nc.dram_tensor(
        "out", [M_SHARD, OUT_DIM], dt_out, kind="ExternalOutput"
    ).ap()
    out_r = out.rearrange("(m p) n -> p m n", p=P)  # [128, MT, OUT_DIM]

    def ev_copy(i, out_ap, in_ap):
        if EV == "vs" and (i % 2 == 1):
            nc.scalar.copy(out_ap, in_ap)
        else:
            nc.vector.tensor_copy(out_ap, in_ap)

    with tile.TileContext(nc) as tc:
        loop_ctx = (
            tc.For_i(0, loop_n, 1,
                     hint_engines=(mybir.EngineType.PE, mybir.EngineType.SP,
                                   mybir.EngineType.DVE,
                                   mybir.EngineType.Activation))
            if loop_n > 1 else contextlib.nullcontext()
        )
        with (
            loop_ctx,
            tc.tile_pool(name="xpool", bufs=XB) as xpool,
            tc.tile_pool(name="wpool", bufs=WB) as wpool,
            tc.tile_pool(name="opool", bufs=OB) as opool,
            tc.tile_pool(name="pspool", bufs=PS_BUFS, space="PSUM") as pspool,
        ):
            x_eng = nc.scalar if XRING == "act" else nc.sync
            x_chunks = []
            if XLAYOUT == "k":
                for c in range(XCHUNKS):
                    x_c = xpool.tile([P, KPC * M_SHARD], dt_in,
                                     name=f"x{c}", tag="x")
                    if do_in_dma:
                        x_eng.dma_start(
                            out=x_c[:],
                            in_=xt[:, c * KPC * M_SHARD:(c + 1) * KPC * M_SHARD],
                        )
                    else:
                        nc.vector.memset(x_c[:, 0:1], 0.0)
                    x_chunks.append(x_c)

            def load_x_mc(mc):
                # XLAYOUT == "m": xt packed m-major; one tile covers MCH
                # m-tiles x all k, contiguous per partition.
                x_t = xpool.tile([P, MCH * KT * P], dt_in,
                                 name=f"xm{mc}", tag="x")
                if do_in_dma:
                    nc.sync.dma_start(
                        out=x_t[:],
                        in_=xt[:, mc * MCH * KT * P:(mc + 1) * MCH * KT * P],
                    )
                else:
                    nc.vector.memset(x_t[:, 0:1], 0.0)
                return x_t

            def load_w(b):
                w_b = wpool.tile([P, KT * NB], dt_in, name=f"w{b}", tag="w")
                w_eng = (nc.scalar if WRING == "split" and b % 2 else nc.sync)
                if do_in_dma:
                    w_eng.dma_start(
                        out=w_b[:],
                        in_=w[:, b * KT * NB:(b + 1) * KT * NB],
                    )
                else:
                    nc.vector.memset(w_b[:, 0:1], 0.0)
                return w_b

            if ORDER == "nd":
                assert MCH * GW <= 8
                for bg in range(NBLK // GW):
                    w_tiles = [load_w(bg * GW + g) for g in range(GW)]
                    if not do_compute:
                        continue
                    for mc in range(MT // MCH):
                        x_mc = load_x_mc(mc) if XLAYOUT == "m" else None
                        psums = [
                            [pspool.tile([P, NB], mybir.dt.float32,
                                         name=f"ps{bg}_{mc}_{m}_{g}", tag="ps")
                             for g in range(GW)]
                            for m in range(MCH)
                        ]
                        for k in range(KT):
                            for m in range(MCH):
                                if XLAYOUT == "m":
                                    xs = x_mc[:, (m * KT + k) * P:
                                              (m * KT + k + 1) * P]
                                else:
                                    xc = x_chunks[k // KPC]
                                    moff = ((k % KPC) * M_SHARD
                                            + (mc * MCH + m) * P)
                                    xs = xc[:, moff:moff + P]
                                if LDW:
                                    nc.tensor.ldweights(xs)
                                for g in range(GW):
                                    nc.tensor.matmul(
                                        psums[m][g][:],
                                        xs,
                                        w_tiles[g][:, k * NB:(k + 1) * NB],
                                        start=(k == 0),
                                        stop=(k == KT - 1),
                                    )
                        if do_evict:
                            for m in range(MCH):
                                o_m = opool.tile([P, GW, NB], dt_out,
                                                 name=f"o{bg}_{mc}_{m}",
                                                 tag="o")
                                for g in range(GW):
                                    ev_copy(m * GW + g,
                                            o_m[:, g, :], psums[m][g][:])
                                nc.scalar.dma_start(
                                    out=out_r[:, mc * MCH + m,
                                              bg * GW * NB:(bg + 1) * GW * NB],
                                    in_=o_m[:],
                                )
                continue_build = False  # structure below is for mi/ki
            else:
                continue_build = True

            if continue_build:
                shared_psums = None
                if not do_evict:
                    shared_psums = [
                        pspool.tile([P, NB], mybir.dt.float32,
                                    name=f"sps{m}", tag="ps")
                        for m in range(MT)
                    ]
                for b in range(NBLK):
                    w_b = load_w(b)
                    if not do_compute:
                        continue
                    o_b = None
                    if do_evict:
                        o_b = opool.tile([P, MT, NB], dt_out,
                                         name=f"o{b}", tag="o")
                    if ORDER == "mi":
                        if shared_psums is not None:
                            psums = shared_psums
                        else:
                            psums = [
                                pspool.tile([P, NB], mybir.dt.float32,
                                            name=f"ps{b}_{m}", tag="ps")
                                for m in range(MT)
                            ]
                        for k in range(KT):
                            xc = x_chunks[k // KPC]
                            koff = (k % KPC) * M_SHARD
                            for m in range(MT):
                                nc.tensor.matmul(
                                    psums[m][:],
                                    xc[:, koff + m * P:koff + (m + 1) * P],
                                    w_b[:, k * NB:(k + 1) * NB],
                                    start=(k == 0),
                                    stop=(k == KT - 1),
                                )
                        if do_evict:
                            for m in range(MT):
                                ev_copy(m, o_b[:, m, :], psums[m][:])
                    else:  # "ki": bank-dwell — one psum bank through all k
                        for m in range(MT):
                            ps = pspool.tile([P, NB], mybir.dt.float32,
                                             name=f"ps{b}_{m}", tag="ps")
                            for k in range(KT):
                                xc = x_chunks[k // KPC]
                                koff = (k % KPC) * M_SHARD
                                nc.tensor.matmul(
                                    ps[:],
                                    xc[:, koff + m * P:koff + (m + 1) * P],
                                    w_b[:, k * NB:(k + 1) * NB],
                                    start=(k == 0),
                                    stop=(k == KT - 1),
                                )
                            nc.vector.tensor_copy(o_b[:, m, :], ps[:])
                    if do_evict:
                        nc.scalar.dma_start(
                            out=out_r[:, :, b * NB:(b + 1) * NB],
                            in_=o_b[:],
                        )

    nc.compile()
    return nc


def _get_nc():
    global _cached_nc
    if _cached_nc is None:
        _cached_nc = _build()
    return _cached_nc


def _pack_x_shard(x_shard):
    """[M_SHARD, IN_DIM] -> [128, KT*M_SHARD] partition-major.

    "k" layout: dest[p, k*M_SHARD + m] = x_shard[m, k*128 + p]
    "m" layout: dest[p, (mt*KT + k)*128 + q] = x_shard[mt*128 + q, k*128 + p]
    """
    if XLAYOUT == "m":
        return np.ascontiguousarray(
            x_shard.reshape(MT, P, KT, P).transpose(3, 0, 2, 1).reshape(P, -1)
        )
    return np.ascontiguousarray(
        x_shard.T.reshape(KT, P, M_SHARD).transpose(1, 0, 2).reshape(P, -1)
    )


def _pack_w(w_full):
    """[IN_DIM, OUT_DIM] -> [128, NBLK*KT*NB] partition-major."""
    # dest[p, b*(KT*NB) + k*NB + j] = w_full[k*128 + p, b*NB + j]
    return np.ascontiguousarray(
        w_full.reshape(KT, P, NBLK, NB).transpose(1, 2, 0, 3).reshape(P, -1)
    )


def _make_in_maps(x, rules_outcome):
    np_dt = _np_dt()
    x = np.asarray(x, dtype=np_dt)
    w = np.asarray(rules_outcome, dtype=np_dt)
    assert x.shape == (BATCH, IN_DIM) and w.shape == (IN_DIM, OUT_DIM)
    w_packed = _pack_w(w)
    return [
        {
            "xt": _pack_x_shard(x[i * M_SHARD:(i + 1) * M_SHARD, :]),
            "w": w_packed,
        }
        for i in range(N_CORES)
    ]


def _run(x, rules_outcome, **spmd_kwargs):
    from concourse.bass_utils import run_bass_kernel_spmd

    in_maps = _make_in_maps(x, rules_outcome)
    nc = _get_nc()
    res = run_bass_kernel_spmd(nc, in_maps, core_ids=list(range(N_CORES)),
                               **spmd_kwargs)
    full = np.concatenate(
        [np.asarray(res.results[i]["out"], dtype=np.float32)
         for i in range(N_CORES)], axis=0)
    return full, res


def kernel(x, rules_outcome):
    out, _ = _run(x, rules_outcome)
    return out
